# revision 14
# baseline (speedup 1.0000x reference)
"""LIF (leaky integrate-and-fire, hard reset) spike-train kernel for TRN2.

Problem: x [32, 4096, 256] f32; scan over last (time) axis:
    u = u*0.125 + x_t ; s = (u >= 1) ; u = (1-s)*u
Output: spikes [32, 4096, 256] f32 (0.0/1.0).

v3 design (fused custom DVE op + int16 input + sentinel spike decode +
stall-hiding group interleave + paired ACT/output):

* Data-parallel over the 131072 independent neurons across 8 cores (16384
  each), laid out as [128 partitions x 128 cols]; T=256 split into STAG_B
  staggered blocks (independent free-dim columns at different time offsets),
  FD = STAG_B*128 columns per step.  Blocks 1.. warm up for WARM steps from
  u=0 (error decays 8x/step; validated vs host sim: flip count unchanged
  down to WARM=6).

* Input quantized host-side to int16 (q = rint(x*4096)); device runs the
  recurrence in v = u*4096 space.  Halves input HBM traffic; adds 626 spike
  flips out of 5.1M (rel err 1.1e-2 < 2e-2 gate, deterministic).

* ONE custom DVE op per step (registered into concourse.dve_ops at import;
  single-uop program, streams at 1 elem/cycle):
      z   = (v > SENT) ? v : 0          # absorb last step's reset sentinel
      y   = z*tau + q_t                 # leaky integrate (i16 q converts
      out = (y < 4096) ? y : SENT       #   in the operand datapath)
  SENT = -2^50; the z-clamp restores exact reset-to-0 dynamics next step.

* The per-step column space is split into NGROUP independent chains
  (group g covers cols [g*FD/NGROUP, (g+1)*FD/NGROUP)), issued round-robin:
  consecutive DVE instructions belong to different chains, which hides the
  ~240ns read-after-write stall of a dependent back-to-back chain
  (measured: 775ns -> 621ns per [128,512] op).

* ACT engine emits the spike train from the v state: d = Sign(-2^-40*v - 1)
  in int8 {-1,+1}; d=+1 iff v==SENT iff spiked; d=0 impossible.  v lives in
  [128, 2, FD] step-pair tiles so one ACT instruction (and one output DMA)
  covers 2 steps, amortizing ACT's ~190ns fixed cost.  Host decodes
  spikes = (d == 1).

* DMAs are spread across hardware queues (queue = issuing engine): input
  chunk loads alternate SP/ACT rings, output stores ride the ACT ring.
  One queue drains ~265 GB/s; two in parallel reach the HBM limit
  (measured DMA-only floor: 47.5us one queue -> 41.8us split).

Measured on HW (loop-slope, per 8-core iteration): v1 3-op baseline
65.7us; fused v2 70.4us (RAW-stalled chain); v3 structure 58.5us;
+ queue split 50.8us.  Compute ~46us DVE busy, DMA floor ~42us.
"""

import numpy as np

# ---- problem constants (hardcoded; kernel.py must be self-contained) ----
B_, N_, T_ = 32, 4096, 256
NCORES = 8
NEUR = B_ * N_              # 131072 neurons total
NPC = NEUR // NCORES        # 16384 neurons per core
TAU = 0.125
SCALE = 4096.0              # v = u * SCALE; threshold VTH*SCALE
TH = 4096.0
SENT = -(2.0 ** 50)         # reset sentinel (absorbed to 0 next step)
ASCALE = -(2.0 ** -40)      # spike decode: d = Sign(ASCALE*v - 1)

# ---- kernel configuration ----
STAG_B = 8        # staggered time blocks
WARM = 4          # warmup steps per block (block 0 needs none; 8x decay
                  # per step keeps warm-start flips inside the quantization
                  # noise down to WARM=4, host-validated)
TC = 2            # time-steps per input DMA chunk (must divide L_)
ODMA_K = 2        # output steps per output-DMA (multiple of 2 if ACT_PAIR)
NV = 3            # ring buffers for the v state (pairs if ACT_PAIR)
COMPUTE = 1       # 1 = full; 0 = in+out DMA only; 2 = in-DMA only; 3 = out only
LOOP_K = 0        # benchmark-only: repeat whole body K times (tc.For_i)
NGROUP = 2        # independent column-chain groups per step (stall hiding)
ACT_PAIR = 0      # 1 = one ACT op + one out-DMA per 2 steps
IN_Q = "split"    # input-DMA issuing queue(s): sp | act | pool | split
OUT_Q = "act"     # output-DMA queue: sp | act | pool | split

L_ = T_ // STAG_B           # block length (= steps with output)
FD = STAG_B * 128           # free dim per step
WFD = (STAG_B - 1) * 128    # warmup column count

_cache = {}


def _register_lif_op():
    """Register the fused LIF-step custom DVE op (idempotent).

    Uses the documented dve_ops extension point: append a DveOp to OPS,
    mirror it in CUSTOM_DVE_SPECS/_SUB_OPCODE_FOR_NAME.  The uop table is
    generated per-NEFF client-side (bass_utils.dve_table_for_ops); rows
    [1, 0x20) are free.
    """
    from concourse import dve_ops
    from concourse.dve_spec import (
        C0, C1, C2, Spec, Src0, Src1, Zero, select, lower, _has_src1,
    )
    from concourse.dve_uop import DveOpSpec

    name = "LIF_STEP_ANT"
    for op in dve_ops.OPS:
        if op.name == name:
            return op

    z = select(Src0 > C2, Src0, Zero)
    y = z * C0 + Src1
    body = select(y < C1, y, C2)

    def _ref(in0, in1, s0, s1, imm2):
        zz = np.where(in0.astype(np.float32) > imm2, in0, 0.0).astype(np.float32)
        yy = (zz * np.float32(s0) + in1.astype(np.float32)).astype(np.float32)
        return np.where(yy < s1, yy, np.float32(imm2)).astype(np.float32)

    spec = Spec(body=body, reference=_ref)
    row = dve_ops._CUSTOM_DVE_ROW_BASE + len(dve_ops.OPS)
    assert row < 0x20, "custom DVE row overflow"
    shas = {}
    for ver in ("v3", "v4"):
        s = DveOpSpec(
            name=name, opcode=row, uops=lower(spec, ver=ver),
            rd1_en=_has_src1(spec),
        )
        shas[ver] = s.sha(ver)
    op = dve_ops.DveOp(name, spec, subdim=False, uops_sha=shas)
    dve_ops.OPS.append(op)
    dve_ops.CUSTOM_DVE_SPECS[name] = spec
    dve_ops._SUB_OPCODE_FOR_NAME[name] = row
    return op


def _build_nc():
    import concourse.mybir as mybir
    from concourse.bacc import Bacc
    from concourse.tile import TileContext
    import contextlib

    lif = _register_lif_op()

    nc = Bacc(None, target_bir_lowering=False)
    f32 = mybir.dt.float32
    i16 = mybir.dt.int16
    i8 = mybir.dt.int8

    assert L_ % TC == 0
    n_chunks = L_ // TC

    xs = nc.dram_tensor("xs", [128, L_, FD], i16, kind="ExternalInput")
    osd = nc.dram_tensor("os", [128, L_, FD], i8, kind="ExternalOutput")

    # chunks containing the warmup columns (steps L_-WARM .. L_-1) load first
    wc0 = (L_ - WARM) // TC if STAG_B > 1 and WARM > 0 else n_chunks
    load_order = list(range(wc0, n_chunks)) + list(range(0, wc0))

    with TileContext(nc) as tc:
        with (
            tc.tile_pool(name="v", bufs=NV) as vpool,
            tc.tile_pool(name="xw", bufs=1) as xpool,
            tc.tile_pool(name="ow", bufs=1) as opool,
            tc.tile_pool(name="c", bufs=1) as cpool,
        ):
            nbias = cpool.tile([128, 1], f32)
            nc.vector.memset(nbias[:, :], -1.0)

            xw = {
                ci: xpool.tile([128, TC, FD], i16, tag=f"xw{ci}", name=f"xw{ci}")
                for ci in load_order
            }
            ow = {
                ci: opool.tile([128, TC, FD], i8, tag=f"ow{ci}", name=f"ow{ci}")
                for ci in range(n_chunks)
            }

            loop_cm = tc.For_i(0, LOOP_K, 1) if LOOP_K else contextlib.nullcontext()
            with loop_cm:
                _emit_body(
                    nc, tc, mybir, lif, xs, osd, xw, ow, vpool, nbias, n_chunks
                )
    nc.finalize()
    return nc


def _emit_body(nc, tc, mybir, lif, xs, osd, xw, ow, vpool, nbias, n_chunks):
    f32 = mybir.dt.float32
    Act = mybir.ActivationFunctionType
    load_order = list(xw.keys())

    def _issuer(which, idx):
        mode = IN_Q if which == "in" else OUT_Q
        if mode == "split":
            return (nc.sync, nc.scalar)[idx % 2]
        if mode == "splitp":
            return (nc.sync, nc.gpsimd)[idx % 2]
        return {"sp": nc.sync, "act": nc.scalar, "pool": nc.gpsimd}[mode]

    if COMPUTE != 3:
        for k, ci in enumerate(load_order):
            _issuer("in", k).dma_start(
                out=xw[ci][:, :, :], in_=xs[:, ci * TC : (ci + 1) * TC, :]
            )

    if COMPUTE in (0, 2, 3):
        if COMPUTE in (0, 3):
            for ci in range(n_chunks):
                nc.vector.memset(ow[ci][:, :, :], 0)
                _issuer("out", ci).dma_start(
                    out=osd[:, ci * TC : (ci + 1) * TC, :], in_=ow[ci][:, :, :]
                )
        return

    PAIR = 2 if ACT_PAIR else 1
    GW = FD // NGROUP
    assert not ACT_PAIR or (WARM % 2 == 0 and L_ % 2 == 0 and TC % 2 == 0)

    # v state ring of [128, PAIR, FD] tiles; pre-zero all (warmup writes only
    # cols 128:FD, so block-0 columns must start at the true u=0 state).
    for i in range(NV):
        vb = vpool.tile([128, PAIR, FD], f32, tag="v", name=f"v{i}")
        nc.vector.memset(vb[:, :, :], 0.0)

    state = {"tile": vb, "slot": PAIR - 1, "n": 0}

    def advance():
        """Return (read_tile, read_slot, write_tile, write_slot)."""
        rt, rs = state["tile"], state["slot"]
        if PAIR == 2 and rs == 0:
            wt, ws = rt, 1          # second slot of the same pair tile
        else:
            state["n"] += 1
            wt = vpool.tile(
                [128, PAIR, FD], f32, tag="v", name=f"vs{state['n']}"
            )
            ws = 0
        state["tile"], state["slot"] = wt, ws
        return rt, rs, wt, ws

    if STAG_B > 1 and WARM > 0:
        # Warmup for blocks 1..B-1 (state cols 128:FD), reading x shifted by
        # -128 cols, split into NGROUP interleaved chains to hide RAW stalls.
        wgw = WFD // NGROUP
        wsplit = [
            (128 + g * wgw, (128 + (g + 1) * wgw) if g < NGROUP - 1 else FD)
            for g in range(NGROUP)
        ]
        for tw in range(WARM):
            col = L_ - WARM + tw
            ci, cl = divmod(col, TC)
            rt, rs, wt, ws = advance()
            for c0, c1 in wsplit:
                nc.vector._custom_dve(
                    lif, out=wt[:, ws, c0:c1], in0=rt[:, rs, c0:c1],
                    in1=xw[ci][:, cl, c0 - 128 : c1 - 128],
                    s0=TAU, s1=TH, imm2=SENT,
                )

    for step in range(L_):
        ci, cl = divmod(step, TC)
        rt, rs, wt, ws = advance()
        for g in range(NGROUP):
            c0, c1 = g * GW, (g + 1) * GW
            nc.vector._custom_dve(
                lif, out=wt[:, ws, c0:c1], in0=rt[:, rs, c0:c1],
                in1=xw[ci][:, cl, c0:c1], s0=TAU, s1=TH, imm2=SENT,
            )

        if PAIR == 2:
            if step % 2 == 1:
                # one ACT op for both steps of the pair
                nc.scalar.activation(
                    out=ow[ci][:, cl - 1 : cl + 1, :], in_=wt[:, :, :],
                    func=Act.Sign, scale=ASCALE, bias=nbias[:, :],
                )
        else:
            nc.scalar.activation(
                out=ow[ci][:, cl, :], in_=wt[:, 0, :],
                func=Act.Sign, scale=ASCALE, bias=nbias[:, :],
            )

        if (step + 1) % ODMA_K == 0 or step == L_ - 1:
            g1 = step + 1
            g0 = g1 - (g1 % ODMA_K or ODMA_K)
            c0_, l0 = divmod(g0, TC)
            _issuer("out", g0 // ODMA_K).dma_start(
                out=osd[:, g0:g1, :],
                in_=ow[c0_][:, l0 : l0 + (g1 - g0), :],
            )


def _prep_core_input(xc):
    """xc: [128, 128, 256] (p, g, t) int16 -> xs [128, L_, STAG_B, 128]."""
    # xs[p, step, b, g] = xc[p, g, L_*b + step]
    return np.ascontiguousarray(
        xc.reshape(128, 128, STAG_B, L_).transpose(0, 3, 2, 1)
    )


def _decode_core_output(o):
    """o: [128, L_, FD] int8 -> spikes [16384, 256] f32."""
    o4 = np.asarray(o).reshape(128, L_, STAG_B, 128)
    sp = o4 == 1
    # [p, j, b, g] -> [p, g, b, j] -> [16384, 256]
    return sp.transpose(0, 3, 2, 1).reshape(NPC, T_).astype(np.float32)


def kernel(x, _trace=False):
    from concourse.bass_utils import run_bass_kernel_spmd

    x = np.asarray(x)
    assert x.shape == (B_, N_, T_)
    q = np.clip(
        np.rint(x.astype(np.float32) * np.float32(SCALE)), -32768, 32767
    ).astype(np.int16)
    qf = q.reshape(NEUR, T_)

    in_maps = []
    for c in range(NCORES):
        xc = qf[c * NPC : (c + 1) * NPC].reshape(128, 128, T_)
        in_maps.append({"xs": _prep_core_input(xc)})

    if "nc" not in _cache:
        _cache["nc"] = _build_nc()
    nc = _cache["nc"]

    res = run_bass_kernel_spmd(
        nc, in_maps, core_ids=list(range(NCORES)), trace=_trace
    )
    kernel.last_result = res

    out = np.empty((NEUR, T_), dtype=np.float32)
    for c in range(NCORES):
        out[c * NPC : (c + 1) * NPC] = _decode_core_output(res.results[c]["os"])
    return out.reshape(B_, N_, T_)


kernel.last_result = None
